# revision 47
# baseline (speedup 1.0000x reference)
"""Trainium2 Bass kernel for the SPH composition loss (gnn_message_passing).

Spatial-hash row-sharded strategy: particles are Morton-sorted by spatial
cell; the 6144 query rows form 48 blocks of 128; each of the 8 NeuronCores
gets 6 blocks (size-balanced so all cores run an identical instruction
stream). For each block the pairwise SPH terms are evaluated only against
the particles within h of the block's query set (exact ball-union,
gathered+padded on host). Per-core partial loss sums are combined on host
("all-reduce of the three scalar loss means").

v3 design (feature-GEMM divergence + quarter-scale kernel algebra):
  layout: partitions = seg candidates j, free = seg*128 + query i.
  - d2ps = q^2 + QB from fp16 hi/lo GEMM (13 contract rows)   [PE]
  - q   = Sqrt(d2ps)  per psum chunk                          [Act]
  - u   = min(q,1)-1;  v = min(q,.5)-.5                       [DVE TSP 4x]
  - Aq  = Square(0.5*u) = u^2/4                               [Act]
  - B   = v*v                                                 [DVE TT 2x]
  - Gt  = B - Aq   (= -G/4, G = u^2-4v^2)                     [DVE TT]
  - Gq  = Gt / q   (bounded, no reciprocal needed)            [DVE TT div]
  - cuq = Aq*u (= u^3/4);  cv = B*v (= v^3)                   [Pool STT]
  - s1 (rho): PE streams cuq (wt -4tsv) + cv (wt +4tsv) into
    s1acc6[6,128] psum stripes (block b -> partition b).
  - s2 (div): per-seg feature GEMM fw[128,8]^T x Gq -> fGq48[48,128]
    psum (block b -> partitions 8b..8b+8).  dot_ij never materialized:
    sum_j Gq_ij*dot_ij = sum_k qc[8b+k,i]*fGq[8b+k,i]  (mul48 + ones-MM).
  - tail: zb2 = s1acc6 - 1, abs-reduces, partition_all_reduce -> out.
Reps for timing run inside a tc.For_i hardware loop (NEFF size constant).
"""
import sys
import os
sys.path.insert(0, "/opt/trn_rl_repo")
import numpy as np
from contextlib import ExitStack, nullcontext

NCORES = 8
BQ = 128            # queries per block
GRID = 9            # spatial grid per axis (cell side 1/9 >= h=0.11)
QB = 1e-4           # bias on q^2 (hi/lo-split GEMM noise is ~2e-5)
PAD_X = 100.0       # padding coord in local/h units (q ~ 170 >> 1)
SPB = 4             # segs per d2 psum chunk (4*128 = 512 fp32 = 1 bank)

_PROGRAM_CACHE = {}
_last_results = None
OPTS = {
    "gq_mode": "recip",   # div_dve | div_pool | recip
    "cuq_eng": "gpsimd",
    "cv_eng": "gpsimd",
    "cv_dve_pairs": 1,      # first k pairs' cv computed on DVE instead
    "b_eng": "scalar",
    "gt_eng": "vector",
    "psmm_bufs": 4,
    "wp_bufs": 3,
    "stream_delay": 1,
    "mul48_f32": True,
}


# ---------------------------------------------------------------- host prep
def _morton3(c):
    out = np.zeros(len(c), dtype=np.int64)
    for b in range(4):
        for d in range(3):
            out |= ((c[:, d] >> b) & 1) << (3 * b + d)
    return out


def _build_structure(pos, h):
    """Balanced recursive bisection (spatially compact equal blocks of BQ)
    + per-block exact candidate lists (ball union)."""
    N = pos.shape[0]

    def _bisect(idx, splits):
        if not splits:
            return [idx]
        k = splits[0]
        p = pos[idx]
        ax = int(np.argmax(p.max(0) - p.min(0)))
        srt = idx[np.argsort(p[:, ax], kind="stable")]
        n = len(idx) // k
        return [blk for i in range(k)
                for blk in _bisect(srt[i * n:(i + 1) * n], splits[1:])]

    nblk = N // BQ
    splits = []
    r = nblk
    while r % 2 == 0:
        splits.append(2)
        r //= 2
    if r > 1:
        # odd factor late-but-not-last measures best (tightest ball unions)
        splits.insert(max(len(splits) - 1, 0), r)
    perm = np.concatenate(_bisect(np.arange(N), splits))
    pos_s = pos[perm]
    nblk = N // BQ
    cand_lists = []
    try:
        from scipy.spatial import cKDTree
        tree_all = cKDTree(pos_s)
        for b in range(nblk):
            qt = cKDTree(pos_s[b * BQ:(b + 1) * BQ])
            idx = qt.query_ball_tree(tree_all, r=float(h) * (1 + 1e-6))
            s = set()
            for lst in idx:
                s.update(lst)
            cand_lists.append(np.array(sorted(s), dtype=np.int64))
    except ImportError:
        rr = (float(h) * (1 + 1e-6)) ** 2
        for b in range(nblk):
            qp = pos_s[b * BQ:(b + 1) * BQ]
            d2 = ((qp[:, None, :] - pos_s[None, :, :]) ** 2).sum(-1)
            cand_lists.append(np.nonzero((d2 <= rr).any(axis=0))[0]
                              .astype(np.int64))
    return perm, cand_lists


# ---------------------------------------------------------------- program
def _build_program(nseg_list, h, vol, n_rows_core, reps=1):
    import concourse.bass as bass
    import concourse.tile as tile
    from concourse import bacc, mybir, bass_isa
    from concourse.alu_op_type import AluOpType as alu
    from concourse.tile_rust import add_dep_helper

    f32 = mybir.dt.float32
    f16 = mybir.dt.float16
    AF = mybir.ActivationFunctionType

    h = float(h)
    vol = float(vol)

    nblk = len(nseg_list)
    NQ = nblk * BQ
    assert NQ == n_rows_core
    l1w = (n_rows_core * 3) // 128
    nseg_max = max(nseg_list)

    nc = bacc.Bacc("TRN2", target_bir_lowering=False, debug=False,
                   num_devices=NCORES)
    # All activations used (Sqrt, Square, Abs, Copy, Identity) live in
    # sqrt_and_others; empty the other cached table sets so the first-fit
    # picker resolves everything to one table -> a single table load.
    from concourse.hw_specs import get_activation_tables
    _tabs = get_activation_tables(nc.m.arch)
    if "sqrt_and_others" in _tabs:
        for _k in list(_tabs.keys()):
            if _k != "sqrt_and_others":
                _tabs[_k] = set()

    nseg_sum = sum(nseg_list)
    Loff = [sum(nseg_list[:b]) * 128 for b in range(nblk)]
    d_lhs_all = nc.dram_tensor("lhs_all", [13, nseg_sum * 128], f16,
                               kind="ExternalInput").ap()
    d_fw_all = nc.dram_tensor("fw_all", [128, 8 * nseg_sum + 2], f16,
                              kind="ExternalInput").ap()
    d_rhs_d2 = nc.dram_tensor("rhs_d2", [13, NQ], f16,
                              kind="ExternalInput").ap()
    d_qc = nc.dram_tensor("qc", [8, NQ], f32,
                          kind="ExternalInput").ap()
    d_ypred = nc.dram_tensor("ypred", [128, 2 * l1w], f32,
                             kind="ExternalInput").ap()
    d_out = nc.dram_tensor("out", [1, 4], f32, kind="ExternalOutput").ap()

    es = ExitStack()
    with tile.TileContext(nc) as tc:
        with es:
            pin = es.enter_context(tc.tile_pool(name="pin", bufs=1))
            wp = es.enter_context(
                tc.tile_pool(name="wp", bufs=OPTS["wp_bufs"]))
            tail = es.enter_context(tc.tile_pool(name="tail", bufs=1))
            psmm = es.enter_context(
                tc.tile_pool(name="psmm", bufs=OPTS["psmm_bufs"],
                             space=bass.MemorySpace.PSUM))
            psacc = es.enter_context(
                tc.tile_pool(name="psacc", bufs=1, space=bass.MemorySpace.PSUM))

            cuq_eng = getattr(nc, OPTS["cuq_eng"])
            cv_eng = getattr(nc, OPTS["cv_eng"])
            b_eng = getattr(nc, OPTS["b_eng"])
            gt_eng = getattr(nc, OPTS["gt_eng"])

            # ---- input loads ----
            rhs_d2 = pin.tile([13, NQ], f16, tag="rhs_d2")
            qc = pin.tile([8, NQ], f32, tag="qc")

            # per-pair chunked loads: pair 0's GEMMs can start as soon
            # as its slice lands instead of waiting for the full tensor
            lhs_all = pin.tile([13, nseg_sum * 128], f16, tag="lhs_all")
            fw_all = pin.tile([128, 8 * nseg_sum + 2], f16, tag="fw_all")
            pb = [0] + [Loff[min(2 * p + 2, nblk - 1)] if 2 * p + 2 < nblk
                        else nseg_sum * 128 for p in range(nblk // 2)]
            nc.sync.dma_start(rhs_d2[:], d_rhs_d2)
            for p in range(nblk // 2):
                a, bnd = pb[p], pb[p + 1]
                nc.sync.dma_start(lhs_all[:, a:bnd], d_lhs_all[:, a:bnd])
                fa, fb = a // 16, bnd // 16
                if p == nblk // 2 - 1:
                    fb = 8 * nseg_sum + 2
                nc.sync.dma_start(fw_all[:, fa:fb], d_fw_all[:, fa:fb])
            W_CUQ = fw_all[:, 8 * nseg_sum:8 * nseg_sum + 1]   # -4*tsv
            W_CV = fw_all[:, 8 * nseg_sum + 1:8 * nseg_sum + 2]  # +4*tsv
            lhs_d2_sb = [lhs_all[:, Loff[b]:Loff[b] + nseg_list[b] * 128]
                         for b in range(nblk)]
            fw_sb = [fw_all[:, (Loff[b] // 16):(Loff[b] // 16)
                            + 8 * nseg_list[b]] for b in range(nblk)]

            nc.sync.dma_start(qc[:], d_qc)
            ypred = pin.tile([128, 2 * l1w], f32, tag="ypred")
            nc.sync.dma_start(ypred[:], d_ypred)
            y_sb = ypred[:, 0:l1w]
            pred_sb = ypred[:, l1w:2 * l1w]

            out_sb = tail.tile([1, 4], f32, tag="osb")
            nc.gpsimd.memset(out_sb[:], 0.0)
            # first Activation sits outside the rep loop so the act-table
            # load is not re-executed every iteration
            nc.scalar.activation(out_sb[0:1, 3:4], out_sb[0:1, 3:4], AF.Copy)

            Fb_max = nseg_max * BQ
            mdt = f32 if OPTS["mul48_f32"] else f16

            # column-grouped psum accumulators (base partition 0; a psum
            # tile column may not exceed one 2KB bank -> <=512 f32 cols)
            groups = [(g0, min(g0 + 4, nblk)) for g0 in range(0, nblk, 4)]
            s1ps, fGqps, mulg, zb2sc, zb3sc = [], [], [], [], []
            for gi, (b0, b1) in enumerate(groups):
                W = (b1 - b0) * BQ
                s1ps.append(psacc.tile([1, W], f32, tag=f"s1ps{gi}", name=f"s1ps{gi}"))
                fGqps.append(psacc.tile([8, W], f32, tag=f"fGqps{gi}", name=f"fGqps{gi}"))
                mulg.append(tail.tile([8, W], mdt, tag=f"mul{gi}", name=f"mul{gi}"))
                zb2sc.append(tail.tile([1, W], f32, tag=f"zb2sc{gi}", name=f"zb2sc{gi}"))
                zb3sc.append(tail.tile([1, W], f32, tag=f"zb3sc{gi}", name=f"zb3sc{gi}"))
            ones8 = tail.tile([8, 1], mdt, tag="ones8")
            nc.gpsimd.memset(ones8[:], 1.0)
            acc4 = tail.tile([1, 2 * len(groups) + 2], f32, tag="acc4")
            l1sq = tail.tile([128, 1], f32, tag="l1sq")
            l1pr = tail.tile([128, 1], f32, tag="l1pr")
            e_t = tail.tile([128, l1w], f32, tag="e")
            esq = tail.tile([128, l1w], f32, tag="esq")

            # ---- main pass (hw loop for timing reps) ----
            loop_cm = tc.For_i(0, reps, 1) if reps > 1 else nullcontext()
            with loop_cm:
                # rep counter in out[0,3]: proves which NEFF actually ran
                nc.scalar.activation(out_sb[0:1, 3:4], out_sb[0:1, 3:4],
                                     AF.Identity, bias=1.0)
                # loss1
                nc.vector.tensor_tensor(e_t[:], y_sb[:], pred_sb[:],
                                        alu.subtract)
                nc.scalar.activation(esq[:], e_t[:], AF.Square,
                                     accum_out=l1sq[:])
                nc.gpsimd.partition_all_reduce(l1pr[:], l1sq[:], 128,
                                               bass_isa.ReduceOp.add)
                nc.scalar.activation(out_sb[0:1, 0:1], l1pr[0:1, 0:1],
                                     AF.Copy)

                # paired-block packing with delayed PE reduce streams:
                # two blocks share one contiguous elementwise tile
                # [0:FbA+FbB] (halves elementwise instruction count); pair
                # p's stream MMs are emitted after pair p+D's compute so
                # the in-order PE queue never stalls on the elementwise
                # chain.
                D = OPTS["stream_delay"]
                pairs = [(2 * p, 2 * p + 1) for p in range(nblk // 2)]
                saved = {}

                qsaved = {}

                def emit_d2(p):
                    bA, bB = pairs[p]
                    q_w = wp.tile([128, 2 * Fb_max], f16, tag="q",
                                  name="q_w")
                    qsaved[p] = q_w
                    flat = [(bA, s) for s in range(nseg_list[bA])] + \
                           [(bB, s) for s in range(nseg_list[bB])]
                    for c0 in range(0, len(flat), SPB):
                        chunk = flat[c0:c0 + SPB]
                        cw = len(chunk) * BQ
                        d2ps = psmm.tile([128, SPB * BQ], f32,
                                         tag="d2ps", name="d2ps")
                        prev = None
                        for ci, (b, s) in enumerate(chunk):
                            rq = slice(b * BQ, (b + 1) * BQ)
                            mm = nc.tensor.matmul(
                                d2ps[:, ci * BQ:(ci + 1) * BQ],
                                lhs_d2_sb[b][:, s * 128:(s + 1) * 128],
                                rhs_d2[:, rq], start=(ci == 0),
                                stop=(ci == len(chunk) - 1))
                            if prev is not None:
                                add_dep_helper(mm.ins, prev.ins,
                                               sync=False,
                                               reason="psum group order")
                            prev = mm
                        nc.scalar.activation(q_w[:, c0 * BQ:c0 * BQ + cw],
                                             d2ps[:, :cw], AF.Sqrt)

                def emit_elem(p):
                    bA, bB = pairs[p]
                    offs = {bA: 0, bB: nseg_list[bA] * BQ}
                    Ftot = (nseg_list[bA] + nseg_list[bB]) * BQ
                    q_w = qsaved.pop(p)
                    u_w = wp.tile([128, 2 * Fb_max], f16, tag="u",
                                  name="u_w")
                    nc.vector.tensor_scalar(u_w[:, :Ftot], q_w[:, :Ftot],
                                            1.0, 1.0, alu.min, alu.subtract)
                    v_w = wp.tile([128, 2 * Fb_max], f16, tag="v",
                                  name="v_w")
                    nc.vector.tensor_scalar(v_w[:, :Ftot], q_w[:, :Ftot],
                                            0.5, 0.5, alu.min, alu.subtract)
                    iq = wp.tile([128, 2 * Fb_max], f16, tag="iq", name="iq")
                    with nc.allow_low_precision("iq fp16 is plenty here"):
                        nc.vector.reciprocal(iq[:, :Ftot], q_w[:, :Ftot])
                    Aq = wp.tile([128, 2 * Fb_max], f16, tag="Aq", name="Aq")
                    nc.scalar.activation(Aq[:, :Ftot], u_w[:, :Ftot],
                                         AF.Square, scale=0.5)
                    B_w = wp.tile([128, 2 * Fb_max], f16, tag="B",
                                  name="B_w")
                    if OPTS["b_eng"] == "scalar":
                        nc.scalar.activation(B_w[:, :Ftot], v_w[:, :Ftot],
                                             AF.Square)
                    else:
                        b_eng.tensor_tensor(B_w[:, :Ftot], v_w[:, :Ftot],
                                            v_w[:, :Ftot], alu.mult)
                    Gt = wp.tile([128, 2 * Fb_max], f16, tag="Gt", name="Gt")
                    gt_eng.tensor_tensor(Gt[:, :Ftot], B_w[:, :Ftot],
                                         Aq[:, :Ftot], alu.subtract)
                    Gq = wp.tile([128, 2 * Fb_max], f16, tag="Gq", name="Gq")
                    nc.vector.tensor_tensor(Gq[:, :Ftot], Gt[:, :Ftot],
                                            iq[:, :Ftot], alu.mult)
                    cuq = wp.tile([128, 2 * Fb_max], f16, tag="cuq",
                                  name="cuq")
                    cuq_eng.tensor_tensor(cuq[:, :Ftot], Aq[:, :Ftot],
                                          u_w[:, :Ftot], alu.mult)
                    cv = wp.tile([128, 2 * Fb_max], f16, tag="cv", name="cv")
                    cve = nc.vector if p < OPTS["cv_dve_pairs"] else cv_eng
                    cve.tensor_tensor(cv[:, :Ftot], B_w[:, :Ftot],
                                      v_w[:, :Ftot], alu.mult)
                    saved[bA] = (cuq, cv, Gq, 0)
                    saved[bB] = (cuq, cv, Gq, offs[bB])

                def emit_streams(b):
                    ns = nseg_list[b]
                    cuq, cv, Gq, off = saved.pop(b)
                    gi = b // 4
                    b0 = groups[gi][0]
                    cols = slice((b - b0) * BQ, (b - b0 + 1) * BQ)
                    prev = None
                    for s in range(ns):
                        cs = slice(off + s * BQ, off + (s + 1) * BQ)
                        mm = nc.tensor.matmul(s1ps[gi][0:1, cols], W_CUQ,
                                              cuq[:, cs], start=(s == 0),
                                              stop=False)
                        if prev is not None:
                            add_dep_helper(mm.ins, prev.ins, sync=False,
                                           reason="s1 group order")
                        prev = mm
                    for s in range(ns):
                        cs = slice(off + s * BQ, off + (s + 1) * BQ)
                        mm = nc.tensor.matmul(s1ps[gi][0:1, cols], W_CV,
                                              cv[:, cs], start=False,
                                              stop=(s == ns - 1))
                        add_dep_helper(mm.ins, prev.ins, sync=False,
                                       reason="s1 group order")
                        prev = mm
                    prev = None
                    for s in range(ns):
                        cs = slice(off + s * BQ, off + (s + 1) * BQ)
                        mm = nc.tensor.matmul(
                            fGqps[gi][:, cols],
                            fw_sb[b][:, 8 * s:8 * s + 8],
                            Gq[:, cs], start=(s == 0), stop=(s == ns - 1))
                        if prev is not None:
                            add_dep_helper(mm.ins, prev.ins, sync=False,
                                           reason="fGq group order")
                        prev = mm

                # per-group tail emitted as soon as the group's streams
                # are complete (overlaps the remaining pairs' stream burst)
                def tail_group(gi):
                    b0, b1 = groups[gi]
                    qs = slice(b0 * BQ, b1 * BQ)
                    nc.vector.tensor_tensor(mulg[gi][:], fGqps[gi][:],
                                            qc[:, qs], alu.mult)
                    nc.scalar.activation(zb2sc[gi][:], s1ps[gi][:], AF.Abs,
                                         bias=1.0, scale=-1.0,
                                         accum_out=acc4[0:1, 2 * gi:2 * gi + 1])
                    # reuse the s1ps bank for the zb3 partition-reduce MM
                    # (WAR on the abs read above, tracked by the tile deps)
                    nc.tensor.matmul(s1ps[gi][:], ones8[:], mulg[gi][:],
                                     start=True, stop=True)
                    nc.scalar.activation(zb3sc[gi][:], s1ps[gi][:], AF.Abs,
                                         accum_out=acc4[0:1,
                                                        2 * gi + 1:2 * gi + 2])
                npair = len(pairs)
                # last pair index whose streams complete each column group
                glast = {max((b1 - 1) // 2 for b in range(b0, b1)
                             for b1_ in [b1]): gi
                         for gi, (b0, b1) in enumerate(groups)}
                glast = {}
                for gi, (b0, b1) in enumerate(groups):
                    glast.setdefault((b1 - 1) // 2, []).append(gi)
                emit_d2(0)
                for pp in range(npair):
                    if pp + 1 < npair:
                        emit_d2(pp + 1)
                    emit_elem(pp)
                    if pp >= 1:
                        for b in pairs[pp - 1]:
                            emit_streams(b)
                        for gi in glast.get(pp - 1, []):
                            tail_group(gi)
                for b in pairs[npair - 1]:
                    emit_streams(b)
                for gi in glast.get(npair - 1, []):
                    tail_group(gi)


                ng = len(groups)
                nc.vector.tensor_tensor(
                    out_sb[0:1, 1:2], acc4[0:1, 0:1], acc4[0:1, 2:3],
                    alu.add)
                nc.vector.tensor_tensor(
                    out_sb[0:1, 2:3], acc4[0:1, 1:2], acc4[0:1, 3:4],
                    alu.add)

            nc.sync.dma_start(d_out, out_sb[:])
    nc.compile()
    return nc


# ---------------------------------------------------------------- kernel
def prepare(inputs, reps=1):
    """Build (nc, in_maps, N) for the given inputs."""
    pred = np.asarray(inputs["pred"], dtype=np.float32)
    y = np.asarray(inputs["y"], dtype=np.float32)
    mid_pos = np.asarray(inputs["mid_pos"], dtype=np.float32)
    mid_vel = np.asarray(inputs["mid_vel"], dtype=np.float32)
    y_mean = np.asarray(inputs["y_mean"], dtype=np.float32)
    y_std = np.asarray(inputs["y_std"], dtype=np.float32)
    h = float(inputs["h"])
    vol = float(inputs["vol"])
    dt = float(inputs["dt"])
    nb = int(inputs["num_boundary_particles"])
    N = pred.shape[0]
    rows_core = N // NCORES

    y_inv = (y * y_std + y_mean).astype(np.float32)
    pos = mid_pos.copy()
    pos[nb:] += y_inv[nb:]
    vel = mid_vel.copy()
    vel[nb:] += (y_inv[nb:] / dt).astype(np.float32)

    perm, cand_lists = _build_structure(pos, h)
    pos_s = pos[perm]; vel_s = vel[perm]
    y_s = y[perm]; pred_s = pred[perm]

    nblk_total = N // BQ
    nblk_core = nblk_total // NCORES
    # size-balanced slot assignment: slot k gets the k-th octile by size
    order = np.argsort([-len(c) for c in cand_lists], kind="stable")
    slots = [order[k * NCORES:(k + 1) * NCORES] for k in range(nblk_core)]
    nseg_list = []
    for k in range(nblk_core):
        mx = max(len(cand_lists[b]) for b in slots[k])
        nseg_list.append(int(np.ceil(mx / 128)))

    key = (tuple(nseg_list), h, vol, N, reps)
    if key not in _PROGRAM_CACHE:
        _PROGRAM_CACHE[key] = _build_program(nseg_list, h, vol, rows_core,
                                             reps=reps)
    nc = _PROGRAM_CACHE[key]

    sigma = 8.0 / (np.pi * h ** 3)
    tsv = 2.0 * sigma * vol
    ch = -6.0 * sigma * vol / h
    l1w = (rows_core * 3) // 128
    inv_h = 1.0 / h

    in_maps = []
    for c in range(NCORES):
        m = {}
        qsel = []
        rhs_d2 = np.empty((13, rows_core), np.float16)
        qc = np.empty((8, rows_core), np.float32)
        for k in range(nblk_core):
            b = int(slots[k][c])
            qidx = np.arange(b * BQ, (b + 1) * BQ)
            qsel.append(qidx)
            ci = cand_lists[b]
            # block-local, h-scaled coordinates (fp16-friendly ranges)
            cb = pos_s[ci].mean(axis=0)
            vb = vel_s[ci].mean(axis=0)
            cpos = (pos_s[ci] - cb) * inv_h
            cvel = vel_s[ci] - vb
            csq = np.sum(cpos * cpos, axis=1, dtype=np.float64)
            cdiag = np.sum(cpos * cvel, axis=1)
            L = nseg_list[k] * 128
            npad = L - len(ci)
            cpos = np.concatenate([cpos,
                                   np.full((npad, 3), PAD_X, np.float32)])
            cvel = np.concatenate([cvel, np.zeros((npad, 3), np.float32)])
            csq = np.concatenate([csq, np.full(npad, 3 * PAD_X * PAD_X,
                                               np.float64)])
            cdiag = np.concatenate([cdiag, np.zeros(npad, np.float32)])
            # hi/lo fp16 splits: d2 = sqh_j+sql_j+sqh_i+sql_i+QB
            #                        - 2(xh_j.xh_i + xh_j.xl_i + xl_j.xh_i)
            cxh = cpos.astype(np.float16)
            cxl = (cpos - cxh.astype(np.float64)).astype(np.float16)
            csqh = csq.astype(np.float16)
            csql = (csq - csqh.astype(np.float64)).astype(np.float16)
            lhs_d2 = np.empty((13, L), np.float16)
            lhs_d2[0:3] = -2.0 * cxh.T
            lhs_d2[3:6] = -2.0 * cxh.T
            lhs_d2[6:9] = -2.0 * cxl.T
            lhs_d2[9] = csqh
            lhs_d2[10] = csql
            lhs_d2[11] = 1.0
            lhs_d2[12] = 1.0
            m.setdefault("_lhs_parts", []).append(lhs_d2)
            # s2 feature weights: per-seg [128, 8] cols
            # [x_j(3), v_j(3), diag_j, 1]  (x in h units -> ch has 1/h)
            fw = np.empty((128, 8 * nseg_list[k]), np.float16)
            for s in range(nseg_list[k]):
                sl = slice(s * 128, (s + 1) * 128)
                fw[:, 8 * s + 0:8 * s + 3] = cxh[sl]
                fw[:, 8 * s + 3:8 * s + 6] = cvel[sl].astype(np.float16)
                fw[:, 8 * s + 6] = cdiag[sl].astype(np.float16)
                fw[:, 8 * s + 7] = 1.0
            m.setdefault("_fw_parts", []).append(fw)
            # query-side rows in the same local frame
            qpos = (pos_s[qidx] - cb) * inv_h
            qvel = vel_s[qidx] - vb
            qsq = np.sum(qpos * qpos, axis=1, dtype=np.float64)
            qdiag = np.sum(qpos * qvel, axis=1)
            qxh = qpos.astype(np.float16)
            qxl = (qpos - qxh.astype(np.float64)).astype(np.float16)
            qsqh = qsq.astype(np.float16)
            qsql = (qsq - qsqh.astype(np.float64) + QB).astype(np.float16)
            ks = slice(k * BQ, (k + 1) * BQ)
            rhs_d2[0:3, ks] = qxh.T
            rhs_d2[3:6, ks] = qxl.T
            rhs_d2[6:9, ks] = qxh.T
            rhs_d2[9, ks] = 1.0
            rhs_d2[10, ks] = 1.0
            rhs_d2[11, ks] = qsqh
            rhs_d2[12, ks] = qsql
            # qc rows pair with fw cols: [v_i(3), x_i(3), -1, -diag_i]
            qc[0:3, ks] = qvel.T
            qc[3:6, ks] = qxh.T.astype(np.float32)
            qc[6, ks] = -1.0
            qc[7, ks] = -qdiag
        wcol = np.empty((128, 2), np.float32)
        wcol[:, 0] = -4.0 * tsv
        wcol[:, 1] = 4.0 * tsv
        m["lhs_all"] = np.concatenate(m.pop("_lhs_parts"), axis=1)
        m["fw_all"] = np.concatenate(
            m.pop("_fw_parts") + [wcol.astype(np.float16)], axis=1)
        m["rhs_d2"] = rhs_d2
        m["qc"] = qc
        qidx_all = np.concatenate(qsel)
        m["ypred"] = np.concatenate(
            [y_s[qidx_all].reshape(128, l1w),
             pred_s[qidx_all].reshape(128, l1w)], axis=1)
        in_maps.append(m)
    return nc, in_maps, N


def combine(results, N):
    h = _COMB["h"]; vol = _COMB["vol"]
    sigma = 8.0 / (np.pi * h ** 3)
    tsv = 2.0 * sigma * vol
    ch = -6.0 * sigma * vol / h
    parts = np.stack([results[c]["out"][0] for c in range(NCORES)])
    l1 = float(np.sum(parts[:, 0], dtype=np.float64))
    l2 = float(np.sum(parts[:, 1], dtype=np.float64))
    l3 = float(np.sum(parts[:, 2], dtype=np.float64)) * 4.0 * abs(ch)
    total = np.float32(1.0 * l1 / N) + np.float32(0.1) * np.float32(l2 / N) \
        + np.float32(0.1) * np.float32(l3 / N)
    return np.array(total, dtype=np.float32)


_COMB = {"h": 0.11, "vol": 1.0 / 6144.0}


def kernel(**inputs):
    from concourse.bass_utils import run_bass_kernel_spmd
    _COMB["h"] = float(inputs["h"])
    _COMB["vol"] = float(inputs["vol"])
    nc, in_maps, N = prepare(inputs)
    res = run_bass_kernel_spmd(nc, in_maps, core_ids=list(range(NCORES)))
    global _last_results
    _last_results = res
    return combine(res.results, N)


# revision 49
# speedup vs baseline: 1.0189x; 1.0189x over previous
"""Trainium2 Bass kernel for the SPH composition loss (gnn_message_passing).

Spatial-hash row-sharded strategy: particles are Morton-sorted by spatial
cell; the 6144 query rows form 48 blocks of 128; each of the 8 NeuronCores
gets 6 blocks (size-balanced so all cores run an identical instruction
stream). For each block the pairwise SPH terms are evaluated only against
the particles within h of the block's query set (exact ball-union,
gathered+padded on host). Per-core partial loss sums are combined on host
("all-reduce of the three scalar loss means").

v3 design (feature-GEMM divergence + quarter-scale kernel algebra):
  layout: partitions = seg candidates j, free = seg*128 + query i.
  - d2ps = q^2 + QB from fp16 hi/lo GEMM (13 contract rows)   [PE]
  - q   = Sqrt(d2ps)  per psum chunk                          [Act]
  - u   = min(q,1)-1;  v = min(q,.5)-.5                       [DVE TSP 4x]
  - Aq  = Square(0.5*u) = u^2/4                               [Act]
  - B   = v*v                                                 [DVE TT 2x]
  - Gt  = B - Aq   (= -G/4, G = u^2-4v^2)                     [DVE TT]
  - Gq  = Gt / q   (bounded, no reciprocal needed)            [DVE TT div]
  - cuq = Aq*u (= u^3/4);  cv = B*v (= v^3)                   [Pool STT]
  - s1 (rho): PE streams cuq (wt -4tsv) + cv (wt +4tsv) into
    s1acc6[6,128] psum stripes (block b -> partition b).
  - s2 (div): per-seg feature GEMM fw[128,8]^T x Gq -> fGq48[48,128]
    psum (block b -> partitions 8b..8b+8).  dot_ij never materialized:
    sum_j Gq_ij*dot_ij = sum_k qc[8b+k,i]*fGq[8b+k,i]  (mul48 + ones-MM).
  - tail: zb2 = s1acc6 - 1, abs-reduces, partition_all_reduce -> out.
Reps for timing run inside a tc.For_i hardware loop (NEFF size constant).
"""
import sys
import os
sys.path.insert(0, "/opt/trn_rl_repo")
import numpy as np
from contextlib import ExitStack, nullcontext

NCORES = 8
BQ = 128            # queries per block
GRID = 9            # spatial grid per axis (cell side 1/9 >= h=0.11)
QB = 1e-4           # bias on q^2 (hi/lo-split GEMM noise is ~2e-5)
PAD_X = 100.0       # padding coord in local/h units (q ~ 170 >> 1)
SPB = 4             # segs per d2 psum chunk (4*128 = 512 fp32 = 1 bank)

_PROGRAM_CACHE = {}
_last_results = None
OPTS = {
    "gq_mode": "recip",   # div_dve | div_pool | recip
    "cuq_eng": "gpsimd",
    "cv_eng": "gpsimd",
    "cv_dve_pairs": 0,      # first k pairs' cv computed on DVE instead
    "b_eng": "scalar",
    "gt_eng": "vector",
    "psmm_bufs": 4,
    "wp_bufs": 3,
    "stream_delay": 1,
    "mul48_f32": True,
}


# ---------------------------------------------------------------- host prep
def _morton3(c):
    out = np.zeros(len(c), dtype=np.int64)
    for b in range(4):
        for d in range(3):
            out |= ((c[:, d] >> b) & 1) << (3 * b + d)
    return out


def _build_structure(pos, h):
    """Balanced recursive bisection (spatially compact equal blocks of BQ)
    + per-block exact candidate lists (ball union)."""
    N = pos.shape[0]

    def _bisect(idx, splits):
        if not splits:
            return [idx]
        k = splits[0]
        p = pos[idx]
        ax = int(np.argmax(p.max(0) - p.min(0)))
        srt = idx[np.argsort(p[:, ax], kind="stable")]
        n = len(idx) // k
        return [blk for i in range(k)
                for blk in _bisect(srt[i * n:(i + 1) * n], splits[1:])]

    nblk = N // BQ
    splits = []
    r = nblk
    while r % 2 == 0:
        splits.append(2)
        r //= 2
    if r > 1:
        # odd factor late-but-not-last measures best (tightest ball unions)
        splits.insert(max(len(splits) - 1, 0), r)
    perm = np.concatenate(_bisect(np.arange(N), splits))
    pos_s = pos[perm]
    nblk = N // BQ
    cand_lists = []
    try:
        from scipy.spatial import cKDTree
        tree_all = cKDTree(pos_s)
        for b in range(nblk):
            qt = cKDTree(pos_s[b * BQ:(b + 1) * BQ])
            idx = qt.query_ball_tree(tree_all, r=float(h) * (1 + 1e-6))
            s = set()
            for lst in idx:
                s.update(lst)
            cand_lists.append(np.array(sorted(s), dtype=np.int64))
    except ImportError:
        rr = (float(h) * (1 + 1e-6)) ** 2
        for b in range(nblk):
            qp = pos_s[b * BQ:(b + 1) * BQ]
            d2 = ((qp[:, None, :] - pos_s[None, :, :]) ** 2).sum(-1)
            cand_lists.append(np.nonzero((d2 <= rr).any(axis=0))[0]
                              .astype(np.int64))
    return perm, cand_lists


# ---------------------------------------------------------------- program
def _build_program(nseg_list, h, vol, n_rows_core, reps=1):
    import concourse.bass as bass
    import concourse.tile as tile
    from concourse import bacc, mybir, bass_isa
    from concourse.alu_op_type import AluOpType as alu
    from concourse.tile_rust import add_dep_helper

    f32 = mybir.dt.float32
    f16 = mybir.dt.float16
    AF = mybir.ActivationFunctionType

    h = float(h)
    vol = float(vol)

    nblk = len(nseg_list)
    NQ = nblk * BQ
    assert NQ == n_rows_core
    l1w = (n_rows_core * 3) // 128
    nseg_max = max(nseg_list)

    nc = bacc.Bacc("TRN2", target_bir_lowering=False, debug=False,
                   num_devices=NCORES)
    # All activations used (Sqrt, Square, Abs, Copy, Identity) live in
    # sqrt_and_others; empty the other cached table sets so the first-fit
    # picker resolves everything to one table -> a single table load.
    from concourse.hw_specs import get_activation_tables
    _tabs = get_activation_tables(nc.m.arch)
    if "sqrt_and_others" in _tabs:
        for _k in list(_tabs.keys()):
            if _k != "sqrt_and_others":
                _tabs[_k] = set()

    nseg_sum = sum(nseg_list)
    Loff = [sum(nseg_list[:b]) * 128 for b in range(nblk)]
    d_lhs_all = nc.dram_tensor("lhs_all", [13, nseg_sum * 128], f16,
                               kind="ExternalInput").ap()
    d_fw_all = nc.dram_tensor("fw_all", [128, 8 * nseg_sum + 2], f16,
                              kind="ExternalInput").ap()
    d_rhs_d2 = nc.dram_tensor("rhs_d2", [13, NQ], f16,
                              kind="ExternalInput").ap()
    d_qc = nc.dram_tensor("qc", [8, NQ], f32,
                          kind="ExternalInput").ap()
    d_ypred = nc.dram_tensor("ypred", [128, 2 * l1w], f32,
                             kind="ExternalInput").ap()
    d_out = nc.dram_tensor("out", [1, 4], f32, kind="ExternalOutput").ap()

    es = ExitStack()
    with tile.TileContext(nc) as tc:
        with es:
            pin = es.enter_context(tc.tile_pool(name="pin", bufs=1))
            wp = es.enter_context(
                tc.tile_pool(name="wp", bufs=OPTS["wp_bufs"]))
            tail = es.enter_context(tc.tile_pool(name="tail", bufs=1))
            psmm = es.enter_context(
                tc.tile_pool(name="psmm", bufs=OPTS["psmm_bufs"],
                             space=bass.MemorySpace.PSUM))
            psacc = es.enter_context(
                tc.tile_pool(name="psacc", bufs=1, space=bass.MemorySpace.PSUM))

            cuq_eng = getattr(nc, OPTS["cuq_eng"])
            cv_eng = getattr(nc, OPTS["cv_eng"])
            b_eng = getattr(nc, OPTS["b_eng"])
            gt_eng = getattr(nc, OPTS["gt_eng"])

            # ---- input loads ----
            rhs_d2 = pin.tile([13, NQ], f16, tag="rhs_d2")
            qc = pin.tile([8, NQ], f32, tag="qc")

            # per-pair chunked loads: pair 0's GEMMs can start as soon
            # as its slice lands instead of waiting for the full tensor
            lhs_all = pin.tile([13, nseg_sum * 128], f16, tag="lhs_all")
            fw_all = pin.tile([128, 8 * nseg_sum + 2], f16, tag="fw_all")
            pb = [0] + [Loff[min(2 * p + 2, nblk - 1)] if 2 * p + 2 < nblk
                        else nseg_sum * 128 for p in range(nblk // 2)]
            nc.sync.dma_start(rhs_d2[:], d_rhs_d2)
            for p in range(nblk // 2):
                a, bnd = pb[p], pb[p + 1]
                nc.sync.dma_start(lhs_all[:, a:bnd], d_lhs_all[:, a:bnd])
                fa, fb = a // 16, bnd // 16
                if p == nblk // 2 - 1:
                    fb = 8 * nseg_sum + 2
                nc.sync.dma_start(fw_all[:, fa:fb], d_fw_all[:, fa:fb])
            W_CUQ = fw_all[:, 8 * nseg_sum:8 * nseg_sum + 1]   # -4*tsv
            W_CV = fw_all[:, 8 * nseg_sum + 1:8 * nseg_sum + 2]  # +4*tsv
            lhs_d2_sb = [lhs_all[:, Loff[b]:Loff[b] + nseg_list[b] * 128]
                         for b in range(nblk)]
            fw_sb = [fw_all[:, (Loff[b] // 16):(Loff[b] // 16)
                            + 8 * nseg_list[b]] for b in range(nblk)]

            nc.sync.dma_start(qc[:], d_qc)
            ypred = pin.tile([128, 2 * l1w], f32, tag="ypred")
            nc.sync.dma_start(ypred[:], d_ypred)
            y_sb = ypred[:, 0:l1w]
            pred_sb = ypred[:, l1w:2 * l1w]

            out_sb = tail.tile([1, 4], f32, tag="osb")
            nc.gpsimd.memset(out_sb[:], 0.0)
            # first Activation sits outside the rep loop so the act-table
            # load is not re-executed every iteration
            nc.scalar.activation(out_sb[0:1, 3:4], out_sb[0:1, 3:4], AF.Copy)

            Fb_max = nseg_max * BQ
            mdt = f32 if OPTS["mul48_f32"] else f16

            # column-grouped psum accumulators (base partition 0; a psum
            # tile column may not exceed one 2KB bank -> <=512 f32 cols)
            groups = [(g0, min(g0 + 4, nblk)) for g0 in range(0, nblk, 4)]
            s1ps, fGqps, mulg, zb2sc, zb3sc = [], [], [], [], []
            for gi, (b0, b1) in enumerate(groups):
                W = (b1 - b0) * BQ
                s1ps.append(psacc.tile([1, W], f32, tag=f"s1ps{gi}", name=f"s1ps{gi}"))
                fGqps.append(psacc.tile([8, W], f32, tag=f"fGqps{gi}", name=f"fGqps{gi}"))
                mulg.append(tail.tile([8, W], mdt, tag=f"mul{gi}", name=f"mul{gi}"))
                zb2sc.append(tail.tile([1, W], f32, tag=f"zb2sc{gi}", name=f"zb2sc{gi}"))
                zb3sc.append(tail.tile([1, W], f32, tag=f"zb3sc{gi}", name=f"zb3sc{gi}"))
            ones8 = tail.tile([8, 1], mdt, tag="ones8")
            nc.gpsimd.memset(ones8[:], 1.0)
            acc4 = tail.tile([1, 2 * len(groups) + 2], f32, tag="acc4")
            l1sq = tail.tile([128, 1], f32, tag="l1sq")
            l1pr = tail.tile([128, 1], f32, tag="l1pr")
            e_t = tail.tile([128, l1w], f32, tag="e")
            esq = tail.tile([128, l1w], f32, tag="esq")

            # ---- main pass (hw loop for timing reps) ----
            loop_cm = tc.For_i(0, reps, 1) if reps > 1 else nullcontext()
            with loop_cm:
                # rep counter in out[0,3]: proves which NEFF actually ran
                nc.scalar.activation(out_sb[0:1, 3:4], out_sb[0:1, 3:4],
                                     AF.Identity, bias=1.0)
                # loss1
                nc.vector.tensor_tensor(e_t[:], y_sb[:], pred_sb[:],
                                        alu.subtract)
                nc.scalar.activation(esq[:], e_t[:], AF.Square,
                                     accum_out=l1sq[:])

                # paired-block packing with delayed PE reduce streams:
                # two blocks share one contiguous elementwise tile
                # [0:FbA+FbB] (halves elementwise instruction count); pair
                # p's stream MMs are emitted after pair p+D's compute so
                # the in-order PE queue never stalls on the elementwise
                # chain.
                D = OPTS["stream_delay"]
                pairs = [(2 * p, 2 * p + 1) for p in range(nblk // 2)]
                saved = {}

                qsaved = {}

                def emit_d2(p):
                    bA, bB = pairs[p]
                    q_w = wp.tile([128, 2 * Fb_max], f16, tag="q",
                                  name="q_w")
                    qsaved[p] = q_w
                    flat = [(bA, s) for s in range(nseg_list[bA])] + \
                           [(bB, s) for s in range(nseg_list[bB])]
                    for c0 in range(0, len(flat), SPB):
                        chunk = flat[c0:c0 + SPB]
                        cw = len(chunk) * BQ
                        d2ps = psmm.tile([128, SPB * BQ], f32,
                                         tag="d2ps", name="d2ps")
                        prev = None
                        for ci, (b, s) in enumerate(chunk):
                            rq = slice(b * BQ, (b + 1) * BQ)
                            mm = nc.tensor.matmul(
                                d2ps[:, ci * BQ:(ci + 1) * BQ],
                                lhs_d2_sb[b][:, s * 128:(s + 1) * 128],
                                rhs_d2[:, rq], start=(ci == 0),
                                stop=(ci == len(chunk) - 1))
                            if prev is not None:
                                add_dep_helper(mm.ins, prev.ins,
                                               sync=False,
                                               reason="psum group order")
                            prev = mm
                        nc.scalar.activation(q_w[:, c0 * BQ:c0 * BQ + cw],
                                             d2ps[:, :cw], AF.Sqrt)

                def emit_elem(p):
                    bA, bB = pairs[p]
                    offs = {bA: 0, bB: nseg_list[bA] * BQ}
                    Ftot = (nseg_list[bA] + nseg_list[bB]) * BQ
                    q_w = qsaved.pop(p)
                    u_w = wp.tile([128, 2 * Fb_max], f16, tag="u",
                                  name="u_w")
                    nc.vector.tensor_scalar(u_w[:, :Ftot], q_w[:, :Ftot],
                                            1.0, 1.0, alu.min, alu.subtract)
                    v_w = wp.tile([128, 2 * Fb_max], f16, tag="v",
                                  name="v_w")
                    nc.vector.tensor_scalar(v_w[:, :Ftot], q_w[:, :Ftot],
                                            0.5, 0.5, alu.min, alu.subtract)
                    iq = wp.tile([128, 2 * Fb_max], f16, tag="iq", name="iq")
                    with nc.allow_low_precision("iq fp16 is plenty here"):
                        nc.vector.reciprocal(iq[:, :Ftot], q_w[:, :Ftot])
                    Aq = wp.tile([128, 2 * Fb_max], f16, tag="Aq", name="Aq")
                    nc.scalar.activation(Aq[:, :Ftot], u_w[:, :Ftot],
                                         AF.Square, scale=0.5)
                    B_w = wp.tile([128, 2 * Fb_max], f16, tag="B",
                                  name="B_w")
                    if OPTS["b_eng"] == "scalar":
                        nc.scalar.activation(B_w[:, :Ftot], v_w[:, :Ftot],
                                             AF.Square)
                    else:
                        b_eng.tensor_tensor(B_w[:, :Ftot], v_w[:, :Ftot],
                                            v_w[:, :Ftot], alu.mult)
                    Gt = wp.tile([128, 2 * Fb_max], f16, tag="Gt", name="Gt")
                    gt_eng.tensor_tensor(Gt[:, :Ftot], B_w[:, :Ftot],
                                         Aq[:, :Ftot], alu.subtract)
                    Gq = wp.tile([128, 2 * Fb_max], f16, tag="Gq", name="Gq")
                    nc.vector.tensor_tensor(Gq[:, :Ftot], Gt[:, :Ftot],
                                            iq[:, :Ftot], alu.mult)
                    cuq = wp.tile([128, 2 * Fb_max], f16, tag="cuq",
                                  name="cuq")
                    cuq_eng.tensor_tensor(cuq[:, :Ftot], Aq[:, :Ftot],
                                          u_w[:, :Ftot], alu.mult)
                    cv = wp.tile([128, 2 * Fb_max], f16, tag="cv", name="cv")
                    cve = nc.vector if p < OPTS["cv_dve_pairs"] else cv_eng
                    cve.tensor_tensor(cv[:, :Ftot], B_w[:, :Ftot],
                                      v_w[:, :Ftot], alu.mult)
                    saved[bA] = (cuq, cv, Gq, 0)
                    saved[bB] = (cuq, cv, Gq, offs[bB])

                def emit_streams(b):
                    ns = nseg_list[b]
                    cuq, cv, Gq, off = saved.pop(b)
                    gi = b // 4
                    b0 = groups[gi][0]
                    cols = slice((b - b0) * BQ, (b - b0 + 1) * BQ)
                    prev = None
                    for s in range(ns):
                        cs = slice(off + s * BQ, off + (s + 1) * BQ)
                        mm = nc.tensor.matmul(s1ps[gi][0:1, cols], W_CUQ,
                                              cuq[:, cs], start=(s == 0),
                                              stop=False)
                        if prev is not None:
                            add_dep_helper(mm.ins, prev.ins, sync=False,
                                           reason="s1 group order")
                        prev = mm
                    for s in range(ns):
                        cs = slice(off + s * BQ, off + (s + 1) * BQ)
                        mm = nc.tensor.matmul(s1ps[gi][0:1, cols], W_CV,
                                              cv[:, cs], start=False,
                                              stop=(s == ns - 1))
                        add_dep_helper(mm.ins, prev.ins, sync=False,
                                       reason="s1 group order")
                        prev = mm
                    prev = None
                    for s in range(ns):
                        cs = slice(off + s * BQ, off + (s + 1) * BQ)
                        mm = nc.tensor.matmul(
                            fGqps[gi][:, cols],
                            fw_sb[b][:, 8 * s:8 * s + 8],
                            Gq[:, cs], start=(s == 0), stop=(s == ns - 1))
                        if prev is not None:
                            add_dep_helper(mm.ins, prev.ins, sync=False,
                                           reason="fGq group order")
                        prev = mm

                # per-group tail emitted as soon as the group's streams
                # are complete (overlaps the remaining pairs' stream burst)
                def tail_group(gi):
                    b0, b1 = groups[gi]
                    qs = slice(b0 * BQ, b1 * BQ)
                    nc.vector.tensor_tensor(mulg[gi][:], fGqps[gi][:],
                                            qc[:, qs], alu.mult)
                    nc.scalar.activation(zb2sc[gi][:], s1ps[gi][:], AF.Abs,
                                         bias=1.0, scale=-1.0,
                                         accum_out=acc4[0:1, 2 * gi:2 * gi + 1])
                    # reuse the s1ps bank for the zb3 partition-reduce MM
                    # (WAR on the abs read above, tracked by the tile deps)
                    nc.tensor.matmul(s1ps[gi][:], ones8[:], mulg[gi][:],
                                     start=True, stop=True)
                    nc.scalar.activation(zb3sc[gi][:], s1ps[gi][:], AF.Abs,
                                         accum_out=acc4[0:1,
                                                        2 * gi + 1:2 * gi + 2])
                npair = len(pairs)
                # last pair index whose streams complete each column group
                glast = {max((b1 - 1) // 2 for b in range(b0, b1)
                             for b1_ in [b1]): gi
                         for gi, (b0, b1) in enumerate(groups)}
                glast = {}
                for gi, (b0, b1) in enumerate(groups):
                    glast.setdefault((b1 - 1) // 2, []).append(gi)
                emit_d2(0)
                for pp in range(npair):
                    if pp + 1 < npair:
                        emit_d2(pp + 1)
                    emit_elem(pp)
                    if pp >= 1:
                        for b in pairs[pp - 1]:
                            emit_streams(b)
                        for gi in glast.get(pp - 1, []):
                            tail_group(gi)
                for b in pairs[npair - 1]:
                    emit_streams(b)
                for gi in glast.get(npair - 1, []):
                    tail_group(gi)


                nc.gpsimd.partition_all_reduce(l1pr[:], l1sq[:], 128,
                                               bass_isa.ReduceOp.add)
                nc.scalar.activation(out_sb[0:1, 0:1], l1pr[0:1, 0:1],
                                     AF.Copy)
                ng = len(groups)
                nc.vector.tensor_tensor(
                    out_sb[0:1, 1:2], acc4[0:1, 0:1], acc4[0:1, 2:3],
                    alu.add)
                nc.vector.tensor_tensor(
                    out_sb[0:1, 2:3], acc4[0:1, 1:2], acc4[0:1, 3:4],
                    alu.add)

            nc.sync.dma_start(d_out, out_sb[:])
    nc.compile()
    return nc


# ---------------------------------------------------------------- kernel
def prepare(inputs, reps=1):
    """Build (nc, in_maps, N) for the given inputs."""
    pred = np.asarray(inputs["pred"], dtype=np.float32)
    y = np.asarray(inputs["y"], dtype=np.float32)
    mid_pos = np.asarray(inputs["mid_pos"], dtype=np.float32)
    mid_vel = np.asarray(inputs["mid_vel"], dtype=np.float32)
    y_mean = np.asarray(inputs["y_mean"], dtype=np.float32)
    y_std = np.asarray(inputs["y_std"], dtype=np.float32)
    h = float(inputs["h"])
    vol = float(inputs["vol"])
    dt = float(inputs["dt"])
    nb = int(inputs["num_boundary_particles"])
    N = pred.shape[0]
    rows_core = N // NCORES

    y_inv = (y * y_std + y_mean).astype(np.float32)
    pos = mid_pos.copy()
    pos[nb:] += y_inv[nb:]
    vel = mid_vel.copy()
    vel[nb:] += (y_inv[nb:] / dt).astype(np.float32)

    perm, cand_lists = _build_structure(pos, h)
    pos_s = pos[perm]; vel_s = vel[perm]
    y_s = y[perm]; pred_s = pred[perm]

    nblk_total = N // BQ
    nblk_core = nblk_total // NCORES
    # size-balanced slot assignment: slot k gets the k-th octile by size
    order = np.argsort([-len(c) for c in cand_lists], kind="stable")
    slots = [order[k * NCORES:(k + 1) * NCORES] for k in range(nblk_core)]
    nseg_list = []
    for k in range(nblk_core):
        mx = max(len(cand_lists[b]) for b in slots[k])
        nseg_list.append(int(np.ceil(mx / 128)))

    key = (tuple(nseg_list), h, vol, N, reps)
    if key not in _PROGRAM_CACHE:
        _PROGRAM_CACHE[key] = _build_program(nseg_list, h, vol, rows_core,
                                             reps=reps)
    nc = _PROGRAM_CACHE[key]

    sigma = 8.0 / (np.pi * h ** 3)
    tsv = 2.0 * sigma * vol
    ch = -6.0 * sigma * vol / h
    l1w = (rows_core * 3) // 128
    inv_h = 1.0 / h

    in_maps = []
    for c in range(NCORES):
        m = {}
        qsel = []
        rhs_d2 = np.empty((13, rows_core), np.float16)
        qc = np.empty((8, rows_core), np.float32)
        for k in range(nblk_core):
            b = int(slots[k][c])
            qidx = np.arange(b * BQ, (b + 1) * BQ)
            qsel.append(qidx)
            ci = cand_lists[b]
            # block-local, h-scaled coordinates (fp16-friendly ranges)
            cb = pos_s[ci].mean(axis=0)
            vb = vel_s[ci].mean(axis=0)
            cpos = (pos_s[ci] - cb) * inv_h
            cvel = vel_s[ci] - vb
            csq = np.sum(cpos * cpos, axis=1, dtype=np.float64)
            cdiag = np.sum(cpos * cvel, axis=1)
            L = nseg_list[k] * 128
            npad = L - len(ci)
            cpos = np.concatenate([cpos,
                                   np.full((npad, 3), PAD_X, np.float32)])
            cvel = np.concatenate([cvel, np.zeros((npad, 3), np.float32)])
            csq = np.concatenate([csq, np.full(npad, 3 * PAD_X * PAD_X,
                                               np.float64)])
            cdiag = np.concatenate([cdiag, np.zeros(npad, np.float32)])
            # hi/lo fp16 splits: d2 = sqh_j+sql_j+sqh_i+sql_i+QB
            #                        - 2(xh_j.xh_i + xh_j.xl_i + xl_j.xh_i)
            cxh = cpos.astype(np.float16)
            cxl = (cpos - cxh.astype(np.float64)).astype(np.float16)
            csqh = csq.astype(np.float16)
            csql = (csq - csqh.astype(np.float64)).astype(np.float16)
            lhs_d2 = np.empty((13, L), np.float16)
            lhs_d2[0:3] = -2.0 * cxh.T
            lhs_d2[3:6] = -2.0 * cxh.T
            lhs_d2[6:9] = -2.0 * cxl.T
            lhs_d2[9] = csqh
            lhs_d2[10] = csql
            lhs_d2[11] = 1.0
            lhs_d2[12] = 1.0
            m.setdefault("_lhs_parts", []).append(lhs_d2)
            # s2 feature weights: per-seg [128, 8] cols
            # [x_j(3), v_j(3), diag_j, 1]  (x in h units -> ch has 1/h)
            fw = np.empty((128, 8 * nseg_list[k]), np.float16)
            for s in range(nseg_list[k]):
                sl = slice(s * 128, (s + 1) * 128)
                fw[:, 8 * s + 0:8 * s + 3] = cxh[sl]
                fw[:, 8 * s + 3:8 * s + 6] = cvel[sl].astype(np.float16)
                fw[:, 8 * s + 6] = cdiag[sl].astype(np.float16)
                fw[:, 8 * s + 7] = 1.0
            m.setdefault("_fw_parts", []).append(fw)
            # query-side rows in the same local frame
            qpos = (pos_s[qidx] - cb) * inv_h
            qvel = vel_s[qidx] - vb
            qsq = np.sum(qpos * qpos, axis=1, dtype=np.float64)
            qdiag = np.sum(qpos * qvel, axis=1)
            qxh = qpos.astype(np.float16)
            qxl = (qpos - qxh.astype(np.float64)).astype(np.float16)
            qsqh = qsq.astype(np.float16)
            qsql = (qsq - qsqh.astype(np.float64) + QB).astype(np.float16)
            ks = slice(k * BQ, (k + 1) * BQ)
            rhs_d2[0:3, ks] = qxh.T
            rhs_d2[3:6, ks] = qxl.T
            rhs_d2[6:9, ks] = qxh.T
            rhs_d2[9, ks] = 1.0
            rhs_d2[10, ks] = 1.0
            rhs_d2[11, ks] = qsqh
            rhs_d2[12, ks] = qsql
            # qc rows pair with fw cols: [v_i(3), x_i(3), -1, -diag_i]
            qc[0:3, ks] = qvel.T
            qc[3:6, ks] = qxh.T.astype(np.float32)
            qc[6, ks] = -1.0
            qc[7, ks] = -qdiag
        wcol = np.empty((128, 2), np.float32)
        wcol[:, 0] = -4.0 * tsv
        wcol[:, 1] = 4.0 * tsv
        m["lhs_all"] = np.concatenate(m.pop("_lhs_parts"), axis=1)
        m["fw_all"] = np.concatenate(
            m.pop("_fw_parts") + [wcol.astype(np.float16)], axis=1)
        m["rhs_d2"] = rhs_d2
        m["qc"] = qc
        qidx_all = np.concatenate(qsel)
        m["ypred"] = np.concatenate(
            [y_s[qidx_all].reshape(128, l1w),
             pred_s[qidx_all].reshape(128, l1w)], axis=1)
        in_maps.append(m)
    return nc, in_maps, N


def combine(results, N):
    h = _COMB["h"]; vol = _COMB["vol"]
    sigma = 8.0 / (np.pi * h ** 3)
    tsv = 2.0 * sigma * vol
    ch = -6.0 * sigma * vol / h
    parts = np.stack([results[c]["out"][0] for c in range(NCORES)])
    l1 = float(np.sum(parts[:, 0], dtype=np.float64))
    l2 = float(np.sum(parts[:, 1], dtype=np.float64))
    l3 = float(np.sum(parts[:, 2], dtype=np.float64)) * 4.0 * abs(ch)
    total = np.float32(1.0 * l1 / N) + np.float32(0.1) * np.float32(l2 / N) \
        + np.float32(0.1) * np.float32(l3 / N)
    return np.array(total, dtype=np.float32)


_COMB = {"h": 0.11, "vol": 1.0 / 6144.0}


def kernel(**inputs):
    from concourse.bass_utils import run_bass_kernel_spmd
    _COMB["h"] = float(inputs["h"])
    _COMB["vol"] = float(inputs["vol"])
    nc, in_maps, N = prepare(inputs)
    res = run_bass_kernel_spmd(nc, in_maps, core_ids=list(range(NCORES)))
    global _last_results
    _last_results = res
    return combine(res.results, N)


# revision 50
# speedup vs baseline: 1.1286x; 1.1077x over previous
"""Trainium2 Bass kernel for the SPH composition loss (gnn_message_passing).

Spatial-hash row-sharded strategy: particles are Morton-sorted by spatial
cell; the 6144 query rows form 48 blocks of 128; each of the 8 NeuronCores
gets 6 blocks (size-balanced so all cores run an identical instruction
stream). For each block the pairwise SPH terms are evaluated only against
the particles within h of the block's query set (exact ball-union,
gathered+padded on host). Per-core partial loss sums are combined on host
("all-reduce of the three scalar loss means").

v3 design (feature-GEMM divergence + quarter-scale kernel algebra):
  layout: partitions = seg candidates j, free = seg*128 + query i.
  - d2ps = q^2 + QB from fp16 hi/lo GEMM (13 contract rows)   [PE]
  - q   = Sqrt(d2ps)  per psum chunk                          [Act]
  - u   = min(q,1)-1;  v = min(q,.5)-.5                       [DVE TSP 4x]
  - Aq  = Square(0.5*u) = u^2/4                               [Act]
  - B   = v*v                                                 [DVE TT 2x]
  - Gt  = B - Aq   (= -G/4, G = u^2-4v^2)                     [DVE TT]
  - Gq  = Gt / q   (bounded, no reciprocal needed)            [DVE TT div]
  - cuq = Aq*u (= u^3/4);  cv = B*v (= v^3)                   [Pool STT]
  - s1 (rho): PE streams cuq (wt -4tsv) + cv (wt +4tsv) into
    s1acc6[6,128] psum stripes (block b -> partition b).
  - s2 (div): per-seg feature GEMM fw[128,8]^T x Gq -> fGq48[48,128]
    psum (block b -> partitions 8b..8b+8).  dot_ij never materialized:
    sum_j Gq_ij*dot_ij = sum_k qc[8b+k,i]*fGq[8b+k,i]  (mul48 + ones-MM).
  - tail: zb2 = s1acc6 - 1, abs-reduces, partition_all_reduce -> out.
Reps for timing run inside a tc.For_i hardware loop (NEFF size constant).
"""
import sys
import os
sys.path.insert(0, "/opt/trn_rl_repo")
import numpy as np
from contextlib import ExitStack, nullcontext

NCORES = 8
BQ = 128            # queries per block
GRID = 9            # spatial grid per axis (cell side 1/9 >= h=0.11)
QB = 1e-4           # bias on q^2 (hi/lo-split GEMM noise is ~2e-5)
PAD_X = 100.0       # padding coord in local/h units (q ~ 170 >> 1)
SPB = 4             # segs per d2 psum chunk (4*128 = 512 fp32 = 1 bank)

_PROGRAM_CACHE = {}
_last_results = None
OPTS = {
    "gq_mode": "recip",   # div_dve | div_pool | recip
    "cuq_eng": "gpsimd",
    "cv_eng": "gpsimd",
    "cv_dve_pairs": 1,      # first k pairs' cv computed on DVE instead
    "b_eng": "scalar",
    "gt_eng": "vector",
    "psmm_bufs": 4,
    "wp_bufs": 3,
    "stream_delay": 1,
    "mul48_f32": True,
}


# ---------------------------------------------------------------- host prep
def _morton3(c):
    out = np.zeros(len(c), dtype=np.int64)
    for b in range(4):
        for d in range(3):
            out |= ((c[:, d] >> b) & 1) << (3 * b + d)
    return out


def _build_structure(pos, h):
    """Balanced recursive bisection (spatially compact equal blocks of BQ)
    + per-block exact candidate lists (ball union)."""
    N = pos.shape[0]

    def _bisect(idx, splits):
        if not splits:
            return [idx]
        k = splits[0]
        p = pos[idx]
        ax = int(np.argmax(p.max(0) - p.min(0)))
        srt = idx[np.argsort(p[:, ax], kind="stable")]
        n = len(idx) // k
        return [blk for i in range(k)
                for blk in _bisect(srt[i * n:(i + 1) * n], splits[1:])]

    nblk = N // BQ
    splits = []
    r = nblk
    while r % 2 == 0:
        splits.append(2)
        r //= 2
    if r > 1:
        # odd factor late-but-not-last measures best (tightest ball unions)
        splits.insert(max(len(splits) - 1, 0), r)
    perm = np.concatenate(_bisect(np.arange(N), splits))
    pos_s = pos[perm]
    nblk = N // BQ
    cand_lists = []
    try:
        from scipy.spatial import cKDTree
        tree_all = cKDTree(pos_s)
        for b in range(nblk):
            qt = cKDTree(pos_s[b * BQ:(b + 1) * BQ])
            idx = qt.query_ball_tree(tree_all, r=float(h) * (1 + 1e-6))
            s = set()
            for lst in idx:
                s.update(lst)
            cand_lists.append(np.array(sorted(s), dtype=np.int64))
    except ImportError:
        rr = (float(h) * (1 + 1e-6)) ** 2
        for b in range(nblk):
            qp = pos_s[b * BQ:(b + 1) * BQ]
            d2 = ((qp[:, None, :] - pos_s[None, :, :]) ** 2).sum(-1)
            cand_lists.append(np.nonzero((d2 <= rr).any(axis=0))[0]
                              .astype(np.int64))
    return perm, cand_lists


# ---------------------------------------------------------------- program
def _build_program(nseg_list, h, vol, n_rows_core, reps=1):
    import concourse.bass as bass
    import concourse.tile as tile
    from concourse import bacc, mybir, bass_isa
    from concourse.alu_op_type import AluOpType as alu
    from concourse.tile_rust import add_dep_helper

    f32 = mybir.dt.float32
    f16 = mybir.dt.float16
    AF = mybir.ActivationFunctionType

    h = float(h)
    vol = float(vol)

    nblk = len(nseg_list)
    NQ = nblk * BQ
    assert NQ == n_rows_core
    l1w = (n_rows_core * 3) // 128
    nseg_max = max(nseg_list)

    nc = bacc.Bacc("TRN2", target_bir_lowering=False, debug=False,
                   num_devices=NCORES)
    # All activations used (Sqrt, Square, Abs, Copy, Identity) live in
    # sqrt_and_others; empty the other cached table sets so the first-fit
    # picker resolves everything to one table -> a single table load.
    from concourse.hw_specs import get_activation_tables
    _tabs = get_activation_tables(nc.m.arch)
    if "sqrt_and_others" in _tabs:
        for _k in list(_tabs.keys()):
            if _k != "sqrt_and_others":
                _tabs[_k] = set()

    nseg_sum = sum(nseg_list)
    Loff = [sum(nseg_list[:b]) * 128 for b in range(nblk)]
    d_lhs_all = nc.dram_tensor("lhs_all", [13, nseg_sum * 128], f16,
                               kind="ExternalInput").ap()
    d_fw_all = nc.dram_tensor("fw_all", [128, 8 * nseg_sum + 2], f16,
                              kind="ExternalInput").ap()
    d_rhs_d2 = nc.dram_tensor("rhs_d2", [13, NQ], f16,
                              kind="ExternalInput").ap()
    d_qc = nc.dram_tensor("qc", [8, NQ], f32,
                          kind="ExternalInput").ap()
    d_ypred = nc.dram_tensor("ypred", [128, 2 * l1w], f32,
                             kind="ExternalInput").ap()
    d_out = nc.dram_tensor("out", [1, 4], f32, kind="ExternalOutput").ap()

    es = ExitStack()
    with tile.TileContext(nc) as tc:
        with es:
            pin = es.enter_context(tc.tile_pool(name="pin", bufs=1))
            wp = es.enter_context(
                tc.tile_pool(name="wp", bufs=OPTS["wp_bufs"]))
            tail = es.enter_context(tc.tile_pool(name="tail", bufs=1))
            psmm = es.enter_context(
                tc.tile_pool(name="psmm", bufs=OPTS["psmm_bufs"],
                             space=bass.MemorySpace.PSUM))
            psacc = es.enter_context(
                tc.tile_pool(name="psacc", bufs=1, space=bass.MemorySpace.PSUM))

            cuq_eng = getattr(nc, OPTS["cuq_eng"])
            cv_eng = getattr(nc, OPTS["cv_eng"])
            b_eng = getattr(nc, OPTS["b_eng"])
            gt_eng = getattr(nc, OPTS["gt_eng"])

            # ---- input loads ----
            rhs_d2 = pin.tile([13, NQ], f16, tag="rhs_d2")
            qc = pin.tile([8, NQ], f32, tag="qc")

            # per-pair chunked loads: pair 0's GEMMs can start as soon
            # as its slice lands instead of waiting for the full tensor
            lhs_all = pin.tile([13, nseg_sum * 128], f16, tag="lhs_all")
            fw_all = pin.tile([128, 8 * nseg_sum + 2], f16, tag="fw_all")
            pb = [0] + [Loff[min(2 * p + 2, nblk - 1)] if 2 * p + 2 < nblk
                        else nseg_sum * 128 for p in range(nblk // 2)]
            nc.sync.dma_start(rhs_d2[:], d_rhs_d2)
            for p in range(nblk // 2):
                a, bnd = pb[p], pb[p + 1]
                nc.sync.dma_start(lhs_all[:, a:bnd], d_lhs_all[:, a:bnd])
                fa, fb = a // 16, bnd // 16
                if p == nblk // 2 - 1:
                    fb = 8 * nseg_sum + 2
                nc.sync.dma_start(fw_all[:, fa:fb], d_fw_all[:, fa:fb])
            W_CUQ = fw_all[:, 8 * nseg_sum:8 * nseg_sum + 1]   # -4*tsv
            W_CV = fw_all[:, 8 * nseg_sum + 1:8 * nseg_sum + 2]  # +4*tsv
            lhs_d2_sb = [lhs_all[:, Loff[b]:Loff[b] + nseg_list[b] * 128]
                         for b in range(nblk)]
            fw_sb = [fw_all[:, (Loff[b] // 16):(Loff[b] // 16)
                            + 8 * nseg_list[b]] for b in range(nblk)]

            nc.sync.dma_start(qc[:], d_qc)
            ypred = pin.tile([128, 2 * l1w], f32, tag="ypred")
            nc.sync.dma_start(ypred[:], d_ypred)
            y_sb = ypred[:, 0:l1w]
            pred_sb = ypred[:, l1w:2 * l1w]

            out_sb = tail.tile([1, 4], f32, tag="osb")
            nc.gpsimd.memset(out_sb[:], 0.0)
            # first Activation sits outside the rep loop so the act-table
            # load is not re-executed every iteration
            nc.scalar.activation(out_sb[0:1, 3:4], out_sb[0:1, 3:4], AF.Copy)

            Fb_max = nseg_max * BQ
            mdt = f32 if OPTS["mul48_f32"] else f16

            # column-grouped psum accumulators (base partition 0; a psum
            # tile column may not exceed one 2KB bank -> <=512 f32 cols)
            groups = [(g0, min(g0 + 4, nblk)) for g0 in range(0, nblk, 4)]
            s1ps, fGqps, mulg, zb2sc, zb3sc = [], [], [], [], []
            for gi, (b0, b1) in enumerate(groups):
                W = (b1 - b0) * BQ
                s1ps.append(psacc.tile([1, W], f32, tag=f"s1ps{gi}", name=f"s1ps{gi}"))
                fGqps.append(psacc.tile([8, W], f32, tag=f"fGqps{gi}", name=f"fGqps{gi}"))
                mulg.append(tail.tile([8, W], mdt, tag=f"mul{gi}", name=f"mul{gi}"))
                zb2sc.append(tail.tile([1, W], f32, tag=f"zb2sc{gi}", name=f"zb2sc{gi}"))
                zb3sc.append(tail.tile([1, W], f32, tag=f"zb3sc{gi}", name=f"zb3sc{gi}"))
            ones8 = tail.tile([8, 1], mdt, tag="ones8")
            nc.gpsimd.memset(ones8[:], 1.0)
            acc4 = tail.tile([1, 2 * len(groups) + 2], f32, tag="acc4")
            l1sq = tail.tile([128, 1], f32, tag="l1sq")
            l1pr = tail.tile([128, 1], f32, tag="l1pr")
            e_t = tail.tile([128, l1w], f32, tag="e")
            esq = tail.tile([128, l1w], f32, tag="esq")

            # ---- main pass (hw loop for timing reps) ----
            loop_cm = tc.For_i(0, reps, 1) if reps > 1 else nullcontext()
            with loop_cm:
                # rep counter in out[0,3]: proves which NEFF actually ran
                nc.scalar.activation(out_sb[0:1, 3:4], out_sb[0:1, 3:4],
                                     AF.Identity, bias=1.0)
                # loss1
                nc.vector.tensor_tensor(e_t[:], y_sb[:], pred_sb[:],
                                        alu.subtract)
                nc.scalar.activation(esq[:], e_t[:], AF.Square,
                                     accum_out=l1sq[:])

                # paired-block packing with delayed PE reduce streams:
                # two blocks share one contiguous elementwise tile
                # [0:FbA+FbB] (halves elementwise instruction count); pair
                # p's stream MMs are emitted after pair p+D's compute so
                # the in-order PE queue never stalls on the elementwise
                # chain.
                D = OPTS["stream_delay"]
                pairs = [(2 * p, 2 * p + 1) for p in range(nblk // 2)]
                saved = {}

                qsaved = {}

                def emit_d2(p):
                    bA, bB = pairs[p]
                    q_w = wp.tile([128, 2 * Fb_max], f16, tag="q",
                                  name="q_w")
                    qsaved[p] = q_w
                    flat = [(bA, s) for s in range(nseg_list[bA])] + \
                           [(bB, s) for s in range(nseg_list[bB])]
                    for c0 in range(0, len(flat), SPB):
                        chunk = flat[c0:c0 + SPB]
                        cw = len(chunk) * BQ
                        d2ps = psmm.tile([128, SPB * BQ], f32,
                                         tag="d2ps", name="d2ps")
                        prev = None
                        for ci, (b, s) in enumerate(chunk):
                            rq = slice(b * BQ, (b + 1) * BQ)
                            mm = nc.tensor.matmul(
                                d2ps[:, ci * BQ:(ci + 1) * BQ],
                                lhs_d2_sb[b][:, s * 128:(s + 1) * 128],
                                rhs_d2[:, rq], start=(ci == 0),
                                stop=(ci == len(chunk) - 1))
                            if prev is not None:
                                add_dep_helper(mm.ins, prev.ins,
                                               sync=False,
                                               reason="psum group order")
                            prev = mm
                        nc.scalar.activation(q_w[:, c0 * BQ:c0 * BQ + cw],
                                             d2ps[:, :cw], AF.Sqrt)

                def emit_elem(p):
                    bA, bB = pairs[p]
                    offs = {bA: 0, bB: nseg_list[bA] * BQ}
                    Ftot = (nseg_list[bA] + nseg_list[bB]) * BQ
                    q_w = qsaved.pop(p)
                    u_w = wp.tile([128, 2 * Fb_max], f16, tag="u",
                                  name="u_w")
                    nc.vector.tensor_scalar(u_w[:, :Ftot], q_w[:, :Ftot],
                                            1.0, 1.0, alu.min, alu.subtract)
                    v_w = wp.tile([128, 2 * Fb_max], f16, tag="v",
                                  name="v_w")
                    nc.vector.tensor_scalar(v_w[:, :Ftot], q_w[:, :Ftot],
                                            0.5, 0.5, alu.min, alu.subtract)
                    iq = wp.tile([128, 2 * Fb_max], f16, tag="iq", name="iq")
                    with nc.allow_low_precision("iq fp16 is plenty here"):
                        nc.vector.reciprocal(iq[:, :Ftot], q_w[:, :Ftot])
                    Aq = wp.tile([128, 2 * Fb_max], f16, tag="Aq", name="Aq")
                    nc.scalar.activation(Aq[:, :Ftot], u_w[:, :Ftot],
                                         AF.Square, scale=0.5)
                    B_w = wp.tile([128, 2 * Fb_max], f16, tag="B",
                                  name="B_w")
                    if OPTS["b_eng"] == "scalar":
                        nc.scalar.activation(B_w[:, :Ftot], v_w[:, :Ftot],
                                             AF.Square)
                    else:
                        b_eng.tensor_tensor(B_w[:, :Ftot], v_w[:, :Ftot],
                                            v_w[:, :Ftot], alu.mult)
                    Gt = wp.tile([128, 2 * Fb_max], f16, tag="Gt", name="Gt")
                    gt_eng.tensor_tensor(Gt[:, :Ftot], B_w[:, :Ftot],
                                         Aq[:, :Ftot], alu.subtract)
                    Gq = wp.tile([128, 2 * Fb_max], f16, tag="Gq", name="Gq")
                    nc.vector.tensor_tensor(Gq[:, :Ftot], Gt[:, :Ftot],
                                            iq[:, :Ftot], alu.mult)
                    cuq = wp.tile([128, 2 * Fb_max], f16, tag="cuq",
                                  name="cuq")
                    cuq_eng.tensor_tensor(cuq[:, :Ftot], Aq[:, :Ftot],
                                          u_w[:, :Ftot], alu.mult)
                    cv = wp.tile([128, 2 * Fb_max], f16, tag="cv", name="cv")
                    cve = nc.vector if p < OPTS["cv_dve_pairs"] else cv_eng
                    cve.tensor_tensor(cv[:, :Ftot], B_w[:, :Ftot],
                                      v_w[:, :Ftot], alu.mult)
                    saved[bA] = (cuq, cv, Gq, 0)
                    saved[bB] = (cuq, cv, Gq, offs[bB])

                def emit_streams(b):
                    ns = nseg_list[b]
                    cuq, cv, Gq, off = saved.pop(b)
                    gi = b // 4
                    b0 = groups[gi][0]
                    cols = slice((b - b0) * BQ, (b - b0 + 1) * BQ)
                    prev = None
                    for s in range(ns):
                        cs = slice(off + s * BQ, off + (s + 1) * BQ)
                        mm = nc.tensor.matmul(s1ps[gi][0:1, cols], W_CUQ,
                                              cuq[:, cs], start=(s == 0),
                                              stop=False)
                        if prev is not None:
                            add_dep_helper(mm.ins, prev.ins, sync=False,
                                           reason="s1 group order")
                        prev = mm
                    for s in range(ns):
                        cs = slice(off + s * BQ, off + (s + 1) * BQ)
                        mm = nc.tensor.matmul(s1ps[gi][0:1, cols], W_CV,
                                              cv[:, cs], start=False,
                                              stop=(s == ns - 1))
                        add_dep_helper(mm.ins, prev.ins, sync=False,
                                       reason="s1 group order")
                        prev = mm
                    prev = None
                    for s in range(ns):
                        cs = slice(off + s * BQ, off + (s + 1) * BQ)
                        mm = nc.tensor.matmul(
                            fGqps[gi][:, cols],
                            fw_sb[b][:, 8 * s:8 * s + 8],
                            Gq[:, cs], start=(s == 0), stop=(s == ns - 1))
                        if prev is not None:
                            add_dep_helper(mm.ins, prev.ins, sync=False,
                                           reason="fGq group order")
                        prev = mm

                # per-group tail emitted as soon as the group's streams
                # are complete (overlaps the remaining pairs' stream burst)
                def tail_group(gi):
                    b0, b1 = groups[gi]
                    qs = slice(b0 * BQ, b1 * BQ)
                    nc.vector.tensor_tensor(mulg[gi][:], fGqps[gi][:],
                                            qc[:, qs], alu.mult)
                    nc.scalar.activation(zb2sc[gi][:], s1ps[gi][:], AF.Abs,
                                         bias=1.0, scale=-1.0,
                                         accum_out=acc4[0:1, 2 * gi:2 * gi + 1])
                    # reuse the s1ps bank for the zb3 partition-reduce MM
                    # (WAR on the abs read above, tracked by the tile deps)
                    nc.tensor.matmul(s1ps[gi][:], ones8[:], mulg[gi][:],
                                     start=True, stop=True)
                    nc.scalar.activation(zb3sc[gi][:], s1ps[gi][:], AF.Abs,
                                         accum_out=acc4[0:1,
                                                        2 * gi + 1:2 * gi + 2])
                npair = len(pairs)
                # last pair index whose streams complete each column group
                glast = {max((b1 - 1) // 2 for b in range(b0, b1)
                             for b1_ in [b1]): gi
                         for gi, (b0, b1) in enumerate(groups)}
                glast = {}
                for gi, (b0, b1) in enumerate(groups):
                    glast.setdefault((b1 - 1) // 2, []).append(gi)
                emit_d2(0)
                for pp in range(npair):
                    if pp + 1 < npair:
                        emit_d2(pp + 1)
                    emit_elem(pp)
                    if pp >= 1:
                        for b in pairs[pp - 1]:
                            emit_streams(b)
                        for gi in glast.get(pp - 1, []):
                            tail_group(gi)
                for b in pairs[npair - 1]:
                    emit_streams(b)
                for gi in glast.get(npair - 1, []):
                    tail_group(gi)


                nc.gpsimd.partition_all_reduce(l1pr[:], l1sq[:], 128,
                                               bass_isa.ReduceOp.add)
                nc.scalar.activation(out_sb[0:1, 0:1], l1pr[0:1, 0:1],
                                     AF.Copy)
                ng = len(groups)
                nc.vector.tensor_tensor(
                    out_sb[0:1, 1:2], acc4[0:1, 0:1], acc4[0:1, 2:3],
                    alu.add)
                nc.vector.tensor_tensor(
                    out_sb[0:1, 2:3], acc4[0:1, 1:2], acc4[0:1, 3:4],
                    alu.add)

            nc.sync.dma_start(d_out, out_sb[:])
    nc.compile()
    return nc


# ---------------------------------------------------------------- kernel
def prepare(inputs, reps=1):
    """Build (nc, in_maps, N) for the given inputs."""
    pred = np.asarray(inputs["pred"], dtype=np.float32)
    y = np.asarray(inputs["y"], dtype=np.float32)
    mid_pos = np.asarray(inputs["mid_pos"], dtype=np.float32)
    mid_vel = np.asarray(inputs["mid_vel"], dtype=np.float32)
    y_mean = np.asarray(inputs["y_mean"], dtype=np.float32)
    y_std = np.asarray(inputs["y_std"], dtype=np.float32)
    h = float(inputs["h"])
    vol = float(inputs["vol"])
    dt = float(inputs["dt"])
    nb = int(inputs["num_boundary_particles"])
    N = pred.shape[0]
    rows_core = N // NCORES

    y_inv = (y * y_std + y_mean).astype(np.float32)
    pos = mid_pos.copy()
    pos[nb:] += y_inv[nb:]
    vel = mid_vel.copy()
    vel[nb:] += (y_inv[nb:] / dt).astype(np.float32)

    perm, cand_lists = _build_structure(pos, h)
    pos_s = pos[perm]; vel_s = vel[perm]
    y_s = y[perm]; pred_s = pred[perm]

    nblk_total = N // BQ
    nblk_core = nblk_total // NCORES
    # size-balanced slot assignment: slot k gets the k-th octile by size
    order = np.argsort([-len(c) for c in cand_lists], kind="stable")
    slots = [order[k * NCORES:(k + 1) * NCORES] for k in range(nblk_core)]
    nseg_list = []
    for k in range(nblk_core):
        mx = max(len(cand_lists[b]) for b in slots[k])
        nseg_list.append(int(np.ceil(mx / 128)))

    key = (tuple(nseg_list), h, vol, N, reps)
    if key not in _PROGRAM_CACHE:
        _PROGRAM_CACHE[key] = _build_program(nseg_list, h, vol, rows_core,
                                             reps=reps)
    nc = _PROGRAM_CACHE[key]

    sigma = 8.0 / (np.pi * h ** 3)
    tsv = 2.0 * sigma * vol
    ch = -6.0 * sigma * vol / h
    l1w = (rows_core * 3) // 128
    inv_h = 1.0 / h

    in_maps = []
    for c in range(NCORES):
        m = {}
        qsel = []
        rhs_d2 = np.empty((13, rows_core), np.float16)
        qc = np.empty((8, rows_core), np.float32)
        for k in range(nblk_core):
            b = int(slots[k][c])
            qidx = np.arange(b * BQ, (b + 1) * BQ)
            qsel.append(qidx)
            ci = cand_lists[b]
            # block-local, h-scaled coordinates (fp16-friendly ranges)
            cb = pos_s[ci].mean(axis=0)
            vb = vel_s[ci].mean(axis=0)
            cpos = (pos_s[ci] - cb) * inv_h
            cvel = vel_s[ci] - vb
            csq = np.sum(cpos * cpos, axis=1, dtype=np.float64)
            cdiag = np.sum(cpos * cvel, axis=1)
            L = nseg_list[k] * 128
            npad = L - len(ci)
            cpos = np.concatenate([cpos,
                                   np.full((npad, 3), PAD_X, np.float32)])
            cvel = np.concatenate([cvel, np.zeros((npad, 3), np.float32)])
            csq = np.concatenate([csq, np.full(npad, 3 * PAD_X * PAD_X,
                                               np.float64)])
            cdiag = np.concatenate([cdiag, np.zeros(npad, np.float32)])
            # hi/lo fp16 splits: d2 = sqh_j+sql_j+sqh_i+sql_i+QB
            #                        - 2(xh_j.xh_i + xh_j.xl_i + xl_j.xh_i)
            cxh = cpos.astype(np.float16)
            cxl = (cpos - cxh.astype(np.float64)).astype(np.float16)
            csqh = csq.astype(np.float16)
            csql = (csq - csqh.astype(np.float64)).astype(np.float16)
            lhs_d2 = np.empty((13, L), np.float16)
            lhs_d2[0:3] = -2.0 * cxh.T
            lhs_d2[3:6] = -2.0 * cxh.T
            lhs_d2[6:9] = -2.0 * cxl.T
            lhs_d2[9] = csqh
            lhs_d2[10] = csql
            lhs_d2[11] = 1.0
            lhs_d2[12] = 1.0
            m.setdefault("_lhs_parts", []).append(lhs_d2)
            # s2 feature weights: per-seg [128, 8] cols
            # [x_j(3), v_j(3), diag_j, 1]  (x in h units -> ch has 1/h)
            fw = np.empty((128, 8 * nseg_list[k]), np.float16)
            for s in range(nseg_list[k]):
                sl = slice(s * 128, (s + 1) * 128)
                fw[:, 8 * s + 0:8 * s + 3] = cxh[sl]
                fw[:, 8 * s + 3:8 * s + 6] = cvel[sl].astype(np.float16)
                fw[:, 8 * s + 6] = cdiag[sl].astype(np.float16)
                fw[:, 8 * s + 7] = 1.0
            m.setdefault("_fw_parts", []).append(fw)
            # query-side rows in the same local frame
            qpos = (pos_s[qidx] - cb) * inv_h
            qvel = vel_s[qidx] - vb
            qsq = np.sum(qpos * qpos, axis=1, dtype=np.float64)
            qdiag = np.sum(qpos * qvel, axis=1)
            qxh = qpos.astype(np.float16)
            qxl = (qpos - qxh.astype(np.float64)).astype(np.float16)
            qsqh = qsq.astype(np.float16)
            qsql = (qsq - qsqh.astype(np.float64) + QB).astype(np.float16)
            ks = slice(k * BQ, (k + 1) * BQ)
            rhs_d2[0:3, ks] = qxh.T
            rhs_d2[3:6, ks] = qxl.T
            rhs_d2[6:9, ks] = qxh.T
            rhs_d2[9, ks] = 1.0
            rhs_d2[10, ks] = 1.0
            rhs_d2[11, ks] = qsqh
            rhs_d2[12, ks] = qsql
            # qc rows pair with fw cols: [v_i(3), x_i(3), -1, -diag_i]
            qc[0:3, ks] = qvel.T
            qc[3:6, ks] = qxh.T.astype(np.float32)
            qc[6, ks] = -1.0
            qc[7, ks] = -qdiag
        wcol = np.empty((128, 2), np.float32)
        wcol[:, 0] = -4.0 * tsv
        wcol[:, 1] = 4.0 * tsv
        m["lhs_all"] = np.concatenate(m.pop("_lhs_parts"), axis=1)
        m["fw_all"] = np.concatenate(
            m.pop("_fw_parts") + [wcol.astype(np.float16)], axis=1)
        m["rhs_d2"] = rhs_d2
        m["qc"] = qc
        qidx_all = np.concatenate(qsel)
        m["ypred"] = np.concatenate(
            [y_s[qidx_all].reshape(128, l1w),
             pred_s[qidx_all].reshape(128, l1w)], axis=1)
        in_maps.append(m)
    return nc, in_maps, N


def combine(results, N):
    h = _COMB["h"]; vol = _COMB["vol"]
    sigma = 8.0 / (np.pi * h ** 3)
    tsv = 2.0 * sigma * vol
    ch = -6.0 * sigma * vol / h
    parts = np.stack([results[c]["out"][0] for c in range(NCORES)])
    l1 = float(np.sum(parts[:, 0], dtype=np.float64))
    l2 = float(np.sum(parts[:, 1], dtype=np.float64))
    l3 = float(np.sum(parts[:, 2], dtype=np.float64)) * 4.0 * abs(ch)
    total = np.float32(1.0 * l1 / N) + np.float32(0.1) * np.float32(l2 / N) \
        + np.float32(0.1) * np.float32(l3 / N)
    return np.array(total, dtype=np.float32)


_COMB = {"h": 0.11, "vol": 1.0 / 6144.0}


def kernel(**inputs):
    from concourse.bass_utils import run_bass_kernel_spmd
    _COMB["h"] = float(inputs["h"])
    _COMB["vol"] = float(inputs["vol"])
    nc, in_maps, N = prepare(inputs)
    res = run_bass_kernel_spmd(nc, in_maps, core_ids=list(range(NCORES)))
    global _last_results
    _last_results = res
    return combine(res.results, N)
